# revision 11
# baseline (speedup 1.0000x reference)
"""H2GCN forward pass on 8 Trainium2 NeuronCores (Bass/Tile SPMD kernel).

Strategy (1D row-parallel SpMM, restructured):
  - Nodes sharded across 8 cores (1024 rows each). Both adjacencies are
    decomposed exactly as A_norm = diag(dis) @ A01 @ diag(dis) with A01 the
    0/1 edge mask and dis = d^-1/2.  A01 is streamed as fp8 (0 and 1 are
    exact in e4m3) at half the fp16 bytes; the dis scalings fold into the
    fp16 lhsT operands and the PSUM->SBUF copy-outs, so the SpMMs carry NO
    quantization error beyond fp16.
  - conv1: z.T = [A@h; A2@h].T computed feature-major with lhsT = dis_a*h
    (replicated full embed, fp16) x fp8 adjT tiles (mixed-dtype matmul).
  - BatchNorm is absorbed: z_n = z*c + d.  Statistics are reduced with a
    small Shared-output AllGather + local tree sum (lower latency than
    AllReduce), split per conv1 half so half 0's reduce hides under conv1
    half 1.
  - conv2 is ASSOCIATED into the final projection: the U1/U2 JK blocks only
    feed w_fin, and (A @ z_n) @ Wu^T = A @ (z_n @ Wu^T) = A @ Y with Y only
    64 wide.  Each core computes Y for its rows (z_n @ (Wu c)^T), scales by
    dis_a, AllGathers the tiny [8192, 2x64] fp16 Y, and runs the second
    SpMM 64-wide -- 4x less PE work than the naive 512-wide conv2 and a 16x
    smaller collective.  The constant part of z_n (d) contributes
    rs_a x (Wu_a @ d), added as a rank-1 correction after conv2.
  - conv2's adjacency pass is served mostly from SBUF: the hs1/hs2/xTf
    buffers are dead after conv1, so 12 of the 16 MB re-stream is cached
    there during the statistics window.
  - out.T = Wh@h.T + (Wz c)@z.T + (Wz@d + bf) + dis*(A01@Ys) + rs x yda;
    the [64, R] result is written feature-major and transposed on host.
"""

import numpy as np
import ml_dtypes

import concourse.bass as bass
import concourse.mybir as mybir
import concourse.tile as tile
from concourse import bacc
from concourse.bass_utils import run_bass_kernel_spmd

P = 128
NCORES = 8
BN_EPS = 1e-5

F8 = mybir.dt.float8e4
F16 = mybir.dt.float16
F32 = mybir.dt.float32
NPF8 = ml_dtypes.float8_e4m3

FULL_CFG = dict(NT=8192, R=1024)
IN_CH = 512   # input features
H = 256       # hidden
H2 = 512      # 2*H (BN width)
O = 64        # output features
FM = 14       # 7*H/128 JK weight chunks: [h:0-1, z_n:2-5, U1:6-9, U2:10-13]


def build_program(NT, R):
    KT = NT // P           # 64 global node k-tiles
    KT2 = KT // 2          # 32 paired tiles (fp8 stream layout)
    RT = R // P            # 8 local node tiles
    NCH = [(0, 512), (512, 512)]
    HM = H // P            # 2
    H2M = H2 // P          # 4
    INK = IN_CH // P       # 4
    NC1 = 16               # adjp2 tiles cached in hs1's buffer (32 KB)
    NC2 = 32               # adjp1 tiles cached in the 64 KB buffer

    nc = bacc.Bacc("TRN2", target_bir_lowering=False, debug=False,
                   num_devices=NCORES)

    # --- I/O -------------------------------------------------------------
    xTf = nc.dram_tensor("xTf", [IN_CH, NT], F16, kind="ExternalInput")
    xT = nc.dram_tensor("xT", [IN_CH, R], F16, kind="ExternalInput")
    adjp1 = nc.dram_tensor("adjp1", [KT2 * P, 2 * R], F8, kind="ExternalInput")
    adjp2 = nc.dram_tensor("adjp2", [KT2 * P, 2 * R], F8, kind="ExternalInput")
    wTe = nc.dram_tensor("wTe", [IN_CH, H], F16, kind="ExternalInput")
    be = nc.dram_tensor("be", [P, HM], F32, kind="ExternalInput")
    bebc = nc.dram_tensor("bebc", [P, H], F32, kind="ExternalInput")
    wTf = nc.dram_tensor("wTf", [7 * H, O], F16, kind="ExternalInput")
    bff = nc.dram_tensor("bff", [O, 1], F32, kind="ExternalInput")
    gam = nc.dram_tensor("gam", [P, H2M], F32, kind="ExternalInput")
    bet = nc.dram_tensor("bet", [P, H2M], F32, kind="ExternalInput")
    disP1 = nc.dram_tensor("disP1", [P, KT], F32, kind="ExternalInput")
    rdisP = nc.dram_tensor("rdisP", [P, KT], F32, kind="ExternalInput")
    cu1 = nc.dram_tensor("cu1", [P, R], F16, kind="ExternalInput")
    cu2 = nc.dram_tensor("cu2", [P, R], F16, kind="ExternalInput")
    disNM = nc.dram_tensor("disNM", [P, 2 * RT], F32, kind="ExternalInput")
    disRO1 = nc.dram_tensor("disRO1", [O, R], F16, kind="ExternalInput")
    disRO2 = nc.dram_tensor("disRO2", [O, R], F16, kind="ExternalInput")
    rsRO1 = nc.dram_tensor("rsRO1", [O, R], F16, kind="ExternalInput")
    rsRO2 = nc.dram_tensor("rsRO2", [O, R], F16, kind="ExternalInput")
    out = nc.dram_tensor("out", [O, R], F32, kind="ExternalOutput")

    rg = [list(range(NCORES))]

    with tile.TileContext(nc) as tc:
        with (
            tc.tile_pool(name="const", bufs=1) as const,
            tc.tile_pool(name="feat", bufs=1) as feat,
            tc.tile_pool(name="tmp", bufs=2) as tmp,
            tc.tile_pool(name="stream", bufs=8) as stream,
            tc.tile_pool(name="stream2", bufs=6) as stream2,
            tc.tile_pool(name="ps", bufs=1, space="PSUM") as ps,
            tc.tile_pool(name="dram", bufs=1, space="DRAM") as dram,
        ):
            # --- embed-critical constants first --------------------------
            wTe_sb = const.tile([P, INK, H], F16, name="wTe_sb")
            nc.sync.dma_start(wTe_sb[:], wTe.ap().rearrange("(k p) m -> p k m", p=P))
            bebc_sb = const.tile([P, H], F32, name="bebc_sb")
            nc.sync.dma_start(bebc_sb[:], bebc.ap())
            disP1_sb = const.tile([P, KT], F32, name="disP1_sb")
            nc.sync.dma_start(disP1_sb[:], disP1.ap())

            # full x.T in node-chunk groups so the embed starts early
            xTf_t = xTf.ap().rearrange("(k p) n -> p k n", p=P)
            xTf_sb = feat.tile([P, INK, NT], F16, name="xTf_sb", tag="big64")
            XGRP = NT // 8
            for g in range(0, NT, XGRP):
                nc.sync.dma_start(xTf_sb[:, :, g:g + XGRP], xTf_t[:, :, g:g + XGRP])

            # --- remaining constants (after xTf in the queue) ------------
            be_sb = const.tile([P, HM], F32, name="be_sb")
            nc.sync.dma_start(be_sb[:], be.ap())
            xT_sb = feat.tile([P, INK, R], F16, name="xT_sb", tag="xt8")
            nc.sync.dma_start(xT_sb[:], xT.ap().rearrange("(k p) n -> p k n", p=P))
            rdisP_sb = const.tile([P, KT], F32, name="rdisP_sb")
            nc.sync.dma_start(rdisP_sb[:], rdisP.ap())
            cu_sb = [const.tile([P, R], F16, name=f"cu{a}_sb") for a in (0, 1)]
            nc.sync.dma_start(cu_sb[0][:], cu1.ap())
            nc.sync.dma_start(cu_sb[1][:], cu2.ap())
            disNM_sb = const.tile([P, 2 * RT], F32, name="disNM_sb")
            nc.sync.dma_start(disNM_sb[:], disNM.ap())
            gam_sb = const.tile([P, H2M], F32, name="gam_sb")
            nc.sync.dma_start(gam_sb[:], gam.ap())
            bet_sb = const.tile([P, H2M], F32, name="bet_sb")
            nc.sync.dma_start(bet_sb[:], bet.ap())
            wTf_sb = const.tile([P, FM, O], F16, name="wTf_sb")
            nc.sync.dma_start(wTf_sb[:], wTf.ap().rearrange("(k p) m -> p k m", p=P))
            bff_sb = const.tile([O, 1], F32, name="bff_sb")
            nc.sync.dma_start(bff_sb[:], bff.ap())
            disRO_sb = [const.tile([O, R], F16, name=f"disRO{a}_sb") for a in (0, 1)]
            nc.sync.dma_start(disRO_sb[0][:], disRO1.ap())
            nc.sync.dma_start(disRO_sb[1][:], disRO2.ap())
            rsRO_sb = [const.tile([O, R], F16, name=f"rsRO{a}_sb") for a in (0, 1)]
            nc.sync.dma_start(rsRO_sb[0][:], rsRO1.ap())
            nc.sync.dma_start(rsRO_sb[1][:], rsRO2.ap())

            # --- B1: replicated full embed -> hs1 = dis1*relu(x@We.T+b) ---
            hs1 = feat.tile([P, KT, H], F16, name="hs1", tag="h32")
            for k in range(KT):
                hps = ps.tile([P, H], F32, name=f"hps_{k}", tag=f"acc{k % 4}")
                for t in range(INK):
                    nc.tensor.matmul(
                        hps[:],
                        lhsT=xTf_sb[:, t, k * P:(k + 1) * P],
                        rhs=wTe_sb[:, t, :],
                        start=(t == 0), stop=(t == INK - 1),
                    )
                ht = tmp.tile([P, H], F16, name=f"ht_{k}", tag="htmp", bufs=3)
                nc.vector.tensor_tensor(
                    out=ht[:], in0=hps[:], in1=bebc_sb[:],
                    op=mybir.AluOpType.add)
                nc.scalar.activation(
                    ht[:], ht[:], mybir.ActivationFunctionType.Relu)
                nc.vector.tensor_scalar_mul(
                    hs1[:, k, :], ht[:], disP1_sb[:, k:k + 1])

            # --- B2: local embed, feature-major (JK h block) --------------
            hT_sb = feat.tile([P, HM, R], F16, name="hT_sb")
            for m in range(HM):
                for ci, (cs, cw) in enumerate(NCH):
                    eps_t = ps.tile([P, 512], F32, name=f"eps_{m}_{ci}",
                                    tag=f"acc{4 + (m * 2 + ci) % 4}")
                    for t in range(INK):
                        nc.tensor.matmul(
                            eps_t[:, :cw],
                            lhsT=wTe_sb[:, t, m * P:(m + 1) * P],
                            rhs=xT_sb[:, t, cs:cs + cw],
                            start=(t == 0), stop=(t == INK - 1),
                        )
                    nc.scalar.activation(
                        hT_sb[:, m, cs:cs + cw], eps_t[:, :cw],
                        mybir.ActivationFunctionType.Relu,
                        bias=be_sb[:, m:m + 1],
                    )

            # hs2 = (dis2/dis1) * hs1, built into xTf's buffer (freed now)
            hs2 = feat.tile([P, KT, H], F16, name="hs2", tag="big64")
            for k in range(KT):
                nc.vector.tensor_scalar_mul(
                    hs2[:, k, :], hs1[:, k, :], rdisP_sb[:, k:k + 1])

            # --- conv1: zT = [A@h; A2@h].T, mixed fp16 x fp8 --------------
            zT_sb = feat.tile([P, H2M, R], F16, name="zT_sb")
            stat = [tmp.tile([P, 2 * HM], F32, name=f"stat{h}", bufs=1)
                    for h in (0, 1)]
            stag = []
            for half, (src, hs_a) in enumerate(((adjp1, hs1), (adjp2, hs2))):
                zps = {}
                for m in range(HM):
                    for ci in range(2):
                        zps[(m, ci)] = ps.tile(
                            [P, 512], F32, name=f"zps_{half}_{m}_{ci}",
                            tag=f"acc{half * 4 + m * 2 + ci}")
                for k2 in range(KT2):
                    at = stream.tile([P, 2, R], F8, name=f"c1_{half}_{k2}",
                                     tag="adj")
                    nc.sync.dma_start(at[:], src[k2 * P:(k2 + 1) * P, :])
                    for c in range(2):
                        gk = 2 * k2 + c
                        for m in range(HM):
                            for ci, (cs, cw) in enumerate(NCH):
                                nc.tensor.matmul(
                                    zps[(m, ci)][:, :cw],
                                    lhsT=hs_a[:, gk, m * P:(m + 1) * P],
                                    rhs=at[:, c, cs:cs + cw],
                                    start=(gk == 0), stop=(gk == KT - 1),
                                )
                for m in range(HM):
                    for ci, (cs, cw) in enumerate(NCH):
                        nc.vector.tensor_tensor(
                            out=zT_sb[:, half * HM + m, cs:cs + cw],
                            in0=zps[(m, ci)][:, :cw],
                            in1=cu_sb[half][:, cs:cs + cw],
                            op=mybir.AluOpType.mult)
                # BN statistics for this half: [sum_f0, sum_f1, sq_f0, sq_f1],
                # reduced across cores via a small Shared AllGather.
                for m in range(HM):
                    f = half * HM + m
                    sq = tmp.tile([P, R], F16, name=f"sq_{f}", tag="sq", bufs=2)
                    nc.scalar.activation(
                        sq[:], zT_sb[:, f, :], mybir.ActivationFunctionType.Copy,
                        accum_out=stat[half][:, m:m + 1])
                    sq2 = tmp.tile([P, R], F16, name=f"sq2_{f}", tag="sq", bufs=2)
                    nc.scalar.activation(
                        sq2[:], zT_sb[:, f, :],
                        mybir.ActivationFunctionType.Square,
                        accum_out=stat[half][:, HM + m:HM + m + 1])
                ag_in = dram.tile([P, 2 * HM], F32, name=f"sag_in{half}")
                nc.gpsimd.dma_start(ag_in[:], stat[half][:])
                ag_out = dram.tile([NCORES, P, 2 * HM], F32,
                                   name=f"sag_out{half}", addr_space="Shared")
                nc.gpsimd.collective_compute(
                    "AllGather", mybir.AluOpType.bypass, replica_groups=rg,
                    ins=[ag_in.opt()], outs=[ag_out.opt()],
                )
                stag.append(ag_out)

            # --- conv2 adjacency cache into dead hs1/hs2 buffers ----------
            adjc2 = feat.tile([P, NC1, 2, R], F8, name="adjc2", tag="h32")
            for k2 in range(NC1):
                nc.sync.dma_start(adjc2[:, k2, :, :],
                                  adjp2[k2 * P:(k2 + 1) * P, :])
            adjc1 = feat.tile([P, NC2, 2, R], F8, name="adjc1", tag="big64")
            for k2 in range(NC2):
                nc.sync.dma_start(adjc1[:, k2, :, :],
                                  adjp1[k2 * P:(k2 + 1) * P, :])
            at_tail = []
            for k2 in range(NC1, KT2):
                t2 = stream2.tile([P, 2, R], F8, name=f"c2t_{k2}", tag="adj2")
                nc.sync.dma_start(t2[:], adjp2[k2 * P:(k2 + 1) * P, :])
                at_tail.append(t2)

            # --- per-half: reduce stats, BN coefs, partial Ys, gather -----
            c_t = tmp.tile([P, H2M], F32, name="c_t", bufs=1)
            d_t = tmp.tile([P, H2M], F32, name="d_t", bufs=1)
            d16 = tmp.tile([P, H2M], F16, name="d16", bufs=1)
            eps_sb = tmp.tile([P, 1], F32, name="eps_sb", bufs=1)
            nc.vector.memset(eps_sb[:], BN_EPS)
            s0acc = tmp.tile([O, 1], F32, name="s0acc", bufs=1)
            ydacc = [tmp.tile([O, 1], F32, name=f"ydacc_{a}", bufs=1)
                     for a in (0, 1)]
            ys0 = tmp.tile([P, 2, RT, O], F16, name="ys0", bufs=1)
            ys_nm = tmp.tile([P, 2, RT, O], F16, name="ys_nm", bufs=1)
            for h in (0, 1):
                ts = (0, 1) if h == 0 else (2, 3)
                sl = slice(2 * h, 2 * h + 2)
                # tree-sum the gathered statistics
                sgin = tmp.tile([P, NCORES, 2 * HM], F32, name=f"sgin{h}",
                                bufs=1)
                nc.gpsimd.dma_start(
                    sgin[:], stag[h].rearrange("r p f -> p r f"))
                t1 = tmp.tile([P, 4, 2 * HM], F32, name=f"st1_{h}", bufs=1)
                nc.vector.tensor_tensor(
                    out=t1[:], in0=sgin[:, 0:4, :], in1=sgin[:, 4:8, :],
                    op=mybir.AluOpType.add)
                t2 = tmp.tile([P, 2, 2 * HM], F32, name=f"st2_{h}", bufs=1)
                nc.vector.tensor_tensor(
                    out=t2[:], in0=t1[:, 0:2, :], in1=t1[:, 2:4, :],
                    op=mybir.AluOpType.add)
                sg = tmp.tile([P, 2 * HM], F32, name=f"sg_{h}", bufs=1)
                nc.vector.tensor_tensor(
                    out=sg[:], in0=t2[:, 0, :], in1=t2[:, 1, :],
                    op=mybir.AluOpType.add)
                # c = gam*rsqrt(var+eps), d = bet - mean*c
                nc.scalar.mul(sg[:], sg[:], 1.0 / NT)
                msq = tmp.tile([P, HM], F32, name=f"msq{h}", bufs=1)
                nc.vector.tensor_mul(out=msq[:], in0=sg[:, 0:HM],
                                     in1=sg[:, 0:HM])
                cvar = tmp.tile([P, HM], F32, name=f"cvar{h}", bufs=1)
                nc.vector.tensor_tensor(
                    out=cvar[:], in0=sg[:, HM:2 * HM], in1=msq[:],
                    op=mybir.AluOpType.subtract)
                cstd = tmp.tile([P, HM], F32, name=f"cstd{h}", bufs=1)
                nc.scalar.activation(
                    cstd[:], cvar[:], mybir.ActivationFunctionType.Sqrt,
                    bias=eps_sb[:])
                crstd = tmp.tile([P, HM], F32, name=f"crstd{h}", bufs=1)
                nc.vector.reciprocal(crstd[:], cstd[:])
                nc.vector.tensor_mul(out=c_t[:, sl], in0=crstd[:],
                                     in1=gam_sb[:, sl])
                nc.vector.tensor_mul(out=d_t[:, sl], in0=sg[:, 0:HM],
                                     in1=c_t[:, sl])
                nc.vector.tensor_tensor(
                    out=d_t[:, sl], in0=bet_sb[:, sl], in1=d_t[:, sl],
                    op=mybir.AluOpType.subtract)
                nc.vector.tensor_copy(out=d16[:, sl], in_=d_t[:, sl])
                # s0/yda partials (read UNSCALED weight rows)
                sps = ps.tile([O, 1], F32, name=f"sps{h}", tag="acc0")
                for i, t in enumerate(ts):
                    nc.tensor.matmul(
                        sps[:], lhsT=wTf_sb[:, 2 + t, :], rhs=d16[:, t:t + 1],
                        start=(i == 0), stop=(i == 1))
                if h == 0:
                    nc.vector.tensor_copy(out=s0acc[:], in_=sps[:])
                else:
                    nc.vector.tensor_add(out=s0acc[:], in0=s0acc[:],
                                         in1=sps[:])
                for a in (0, 1):
                    ydp = ps.tile([O, 1], F32, name=f"ydps{h}_{a}",
                                  tag="acc1")
                    for i, t in enumerate(ts):
                        nc.tensor.matmul(
                            ydp[:], lhsT=wTf_sb[:, 6 + 4 * a + t, :],
                            rhs=d16[:, t:t + 1],
                            start=(i == 0), stop=(i == 1))
                    if h == 0:
                        nc.vector.tensor_copy(out=ydacc[a][:], in_=ydp[:])
                    else:
                        nc.vector.tensor_add(out=ydacc[a][:], in0=ydacc[a][:],
                                             in1=ydp[:])
                # scale this half's z_n/U1/U2 weight rows by c, in place
                for base in (2, 6, 10):
                    for t in ts:
                        nc.vector.tensor_scalar_mul(
                            wTf_sb[:, base + t, :], wTf_sb[:, base + t, :],
                            c_t[:, t:t + 1])
                # Ys partials: z(half) @ (Wu_a c)^T, node-major
                for a in (0, 1):
                    for nt in range(RT):
                        yp = ps.tile([P, O], F32, name=f"yp{h}_{a}_{nt}",
                                     tag=f"acc{2 + (a * RT + nt) % 2}")
                        for i, t in enumerate(ts):
                            nc.tensor.matmul(
                                yp[:],
                                lhsT=zT_sb[:, t, nt * P:(nt + 1) * P],
                                rhs=wTf_sb[:, 6 + 4 * a + t, :],
                                start=(i == 0), stop=(i == 1))
                        if h == 0:
                            nc.vector.tensor_scalar_mul(
                                ys0[:, a, nt, :], yp[:],
                                disNM_sb[:, a * RT + nt:a * RT + nt + 1])
                        else:
                            yt = tmp.tile([P, O], F16, name=f"yt_{a}_{nt}",
                                          tag="ytmp", bufs=3)
                            nc.vector.tensor_scalar_mul(
                                yt[:], yp[:],
                                disNM_sb[:, a * RT + nt:a * RT + nt + 1])
                            nc.vector.tensor_add(
                                out=ys_nm[:, a, nt, :],
                                in0=ys0[:, a, nt, :], in1=yt[:])

            s0b = tmp.tile([O, 1], F32, name="s0b", bufs=1)
            nc.vector.tensor_add(out=s0b[:], in0=s0acc[:], in1=bff_sb[:])

            ysin = dram.tile([2, R, O], F16, name="ysin")
            nc.gpsimd.dma_start(
                ysin.rearrange("a (nt p) f -> p (a nt) f", p=P), ys_nm[:])
            ysag = dram.tile([NCORES, 2, R, O], F16, name="ysag",
                             addr_space="Shared")
            nc.gpsimd.collective_compute(
                "AllGather", mybir.AluOpType.bypass, replica_groups=rg,
                ins=[ysin.opt()], outs=[ysag.opt()],
            )
            ysf = []
            for a in (0, 1):
                yf = feat.tile([P, NCORES, RT, O], F16, name=f"ysf_{a}",
                               tag=("xt8" if a == 0 else None))
                for r in range(NCORES):
                    nc.gpsimd.dma_start(
                        yf[:, r, :, :],
                        ysag[r, a].rearrange("(nt p) f -> p nt f", p=P))
                ysf.append(yf)

            # --- final projection base (PE-ready before conv2) ------------
            outsb = tmp.tile([O, R], F32, name="outsb", bufs=1)
            for ci, (cs, cw) in enumerate(NCH):
                op_t = ps.tile([O, 512], F32, name=f"ops_{ci}",
                               tag=f"acc{5 + ci}")
                for t in range(2):
                    nc.tensor.matmul(
                        op_t[:, :cw], lhsT=wTf_sb[:, t, :],
                        rhs=hT_sb[:, t, cs:cs + cw],
                        start=(t == 0), stop=False)
                for t in range(H2M):
                    nc.tensor.matmul(
                        op_t[:, :cw], lhsT=wTf_sb[:, 2 + t, :],
                        rhs=zT_sb[:, t, cs:cs + cw],
                        start=False, stop=(t == H2M - 1))
                nc.vector.tensor_scalar_add(
                    outsb[:, cs:cs + cw], op_t[:, :cw], s0b[:])

            # --- conv2: ups_a = A01_a @ Ys_a  (64-wide, one pass) ---------
            ups = {}
            for a in (0, 1):
                for ci in range(2):
                    ups[(a, ci)] = ps.tile(
                        [O, 512], F32, name=f"ups_{a}_{ci}",
                        tag=f"acc{a * 2 + ci}")
            for k2 in range(KT2):
                at1 = adjc1[:, k2, :, :]
                at2 = adjc2[:, k2, :, :] if k2 < NC1 else at_tail[k2 - NC1]
                for c in range(2):
                    gk = 2 * k2 + c
                    for a, at in ((0, at1), (1, at2)):
                        for ci, (cs, cw) in enumerate(NCH):
                            nc.tensor.matmul(
                                ups[(a, ci)][:, :cw],
                                lhsT=ysf[a][:, gk // RT, gk % RT, :],
                                rhs=at[:, c, cs:cs + cw],
                                start=(gk == 0), stop=(gk == KT - 1),
                            )

            # --- add conv2 contributions + rank-1 yda terms, write out ----
            for a in (0, 1):
                for ci, (cs, cw) in enumerate(NCH):
                    va = tmp.tile([O, 512], F32, name=f"va_{a}_{ci}",
                                  tag="vtmp", bufs=2)
                    nc.vector.tensor_tensor(
                        out=va[:], in0=ups[(a, ci)][:, :cw],
                        in1=disRO_sb[a][:, cs:cs + cw],
                        op=mybir.AluOpType.mult)
                    nc.vector.tensor_tensor(
                        out=outsb[:, cs:cs + cw], in0=outsb[:, cs:cs + cw],
                        in1=va[:], op=mybir.AluOpType.add)
            for a in (0, 1):
                ry = tmp.tile([O, R], F32, name=f"ry_{a}", bufs=1)
                nc.vector.tensor_scalar_mul(
                    ry[:], rsRO_sb[a][:], ydacc[a][:])
                nc.vector.tensor_add(out=outsb[:], in0=outsb[:], in1=ry[:])
            nc.sync.dma_start(out.ap(), outsb[:])

    nc.compile()
    return nc


_PROGRAM_CACHE = {}


def _get_program(NT, R):
    key = (NT, R)
    if key not in _PROGRAM_CACHE:
        _PROGRAM_CACHE[key] = build_program(NT, R)
    return _PROGRAM_CACHE[key]


def make_in_maps(inputs, NT, R):
    """Shard full inputs into per-core input maps (host-side, numpy)."""
    KT = NT // P
    KT2 = KT // 2
    RT = R // P
    HM = H // P
    H2M = H2 // P

    x = np.asarray(inputs["x"], np.float32)
    adj = np.asarray(inputs["adj_t"], np.float32)
    adj2 = np.asarray(inputs["adj_t2"], np.float32)
    we = np.asarray(inputs["w_embed"], np.float32)
    be_v = np.asarray(inputs["b_embed"], np.float32)
    gam_v = np.asarray(inputs["bn_gamma"], np.float32)
    bet_v = np.asarray(inputs["bn_beta"], np.float32)
    wf = np.asarray(inputs["w_fin"], np.float32)
    bf = np.asarray(inputs["b_fin"], np.float32)

    # exact 0/1 decomposition of the gcn-normalized adjacencies
    A01 = [(adj != 0), (adj2 != 0)]
    dis = []
    for A in A01:
        d = A.sum(1, dtype=np.float64).astype(np.float32)
        dis.append(np.where(d > 0, 1.0 / np.sqrt(np.maximum(d, 1e-12)), 0.0)
                   .astype(np.float32))
    rdis = np.where(dis[0] > 0, dis[1] / np.maximum(dis[0], 1e-30), 0.0
                    ).astype(np.float32)
    rs = [adj.sum(1), adj2.sum(1)]   # rowsums of the normalized adjacencies

    xTf_h = np.ascontiguousarray(x.T).astype(np.float16)
    wTe_h = np.ascontiguousarray(we.T).astype(np.float16)
    be_h = np.ascontiguousarray(be_v.reshape(HM, P).T).astype(np.float32)
    bebc_h = np.ascontiguousarray(
        np.broadcast_to(be_v[None, :], (P, H))).astype(np.float32)
    wTf_h = np.ascontiguousarray(wf.T).astype(np.float16)
    bff_h = np.ascontiguousarray(bf[:, None]).astype(np.float32)
    gam_h = np.ascontiguousarray(gam_v.reshape(H2M, P).T).astype(np.float32)
    bet_h = np.ascontiguousarray(bet_v.reshape(H2M, P).T).astype(np.float32)
    disP1_h = np.ascontiguousarray(dis[0].reshape(KT, P).T).astype(np.float32)
    rdisP_h = np.ascontiguousarray(rdis.reshape(KT, P).T).astype(np.float32)

    in_maps = []
    for r in range(NCORES):
        rows = slice(r * R, (r + 1) * R)
        adjp = []
        for A in A01:
            aT = A[rows, :].T.astype(NPF8)   # [NT, R] 0/1 fp8
            adjp.append(np.ascontiguousarray(
                aT.reshape(KT2, 2, P, R).transpose(0, 2, 1, 3)
                  .reshape(KT2 * P, 2 * R)))
        cu_h = [np.ascontiguousarray(
            np.broadcast_to(di[rows][None, :], (P, R))).astype(np.float16)
            for di in dis]
        disNM_h = np.ascontiguousarray(np.concatenate(
            [di[rows].reshape(RT, P).T for di in dis], axis=1)
        ).astype(np.float32)
        disRO_h = [np.ascontiguousarray(
            np.broadcast_to(di[rows][None, :], (O, R))).astype(np.float16)
            for di in dis]
        rsRO_h = [np.ascontiguousarray(
            np.broadcast_to(rv[rows][None, :], (O, R))).astype(np.float16)
            for rv in rs]
        in_maps.append({
            "xTf": xTf_h,
            "xT": np.ascontiguousarray(x[rows].T).astype(np.float16),
            "adjp1": adjp[0], "adjp2": adjp[1],
            "wTe": wTe_h, "be": be_h, "bebc": bebc_h, "wTf": wTf_h,
            "bff": bff_h, "gam": gam_h, "bet": bet_h,
            "disP1": disP1_h, "rdisP": rdisP_h,
            "cu1": cu_h[0], "cu2": cu_h[1],
            "disNM": disNM_h,
            "disRO1": disRO_h[0], "disRO2": disRO_h[1],
            "rsRO1": rsRO_h[0], "rsRO2": rsRO_h[1],
        })
    return in_maps


def kernel(**inputs):
    NT, R = FULL_CFG["NT"], FULL_CFG["R"]
    nc = _get_program(NT, R)
    in_maps = make_in_maps(inputs, NT, R)
    res = run_bass_kernel_spmd(nc, in_maps, core_ids=list(range(NCORES)))
    out = np.concatenate(
        [np.ascontiguousarray(res.results[r]["out"].T)
         for r in range(NCORES)], axis=0)
    return out.astype(np.float32)
